# revision 5
# baseline (speedup 1.0000x reference)
"""Trainium2 Bass kernel for nn_CompleteAttention_68418829025814.

Linformer-style windowed attention, restructured for the PE array:
  - window_reverse is folded into a host-side column permutation of E_w/F_w
    (device works entirely in x's native window order) and a host-side
    permutation of the gathered output.
  - k/v are never materialized: k_low = (E @ x) @ Wk^T + const (the E/F
    projections contract over tokens, so x is used in its native layout).
  - only the q path needs x transposed; done on-device via PE transpose mode.
  - all large matmuls run as float32r (full PE rate at moving-dim >= 256);
    the attn@V + softmax-denominator stage uses bf16 col-packed matmuls
    (tile_position col groups are bf16-only), which lands each head's
    denominator partition-aligned with its output for the DVE division.

Sharding: data-parallel over batch; each of the 8 cores gets 4 batches
(256 windows) of x. Small weights are replicated.
"""

import numpy as np

B_TOT = 32
N_CORES = 8
B_PER = B_TOT // N_CORES      # 4 batches per core
N = 3136                      # tokens per batch
NP = 3200                     # padded tokens per batch (6*512 + 128)
C = 192
H = 6
HD = 32
R = 128
WS = 7

_STATE = {}


def _window_perm():
    """n_of_m[m] = spatial index n for window-order position m."""
    hh, ww, i, j = np.meshgrid(
        np.arange(8), np.arange(8), np.arange(7), np.arange(7), indexing="ij"
    )
    m = (hh * 8 + ww) * 49 + i * 7 + j
    n = (hh * 7 + i) * 56 + ww * 7 + j
    n_of_m = np.empty(N, dtype=np.int64)
    n_of_m[m.ravel()] = n.ravel()
    return n_of_m


def _build_bass():
    import concourse.bacc as bacc
    import concourse.mybir as mybir
    from concourse.tile import TileContext

    f32 = mybir.dt.float32
    f32r = mybir.dt.float32r
    bf16 = mybir.dt.bfloat16

    nc = bacc.Bacc("TRN2", target_bir_lowering=False, debug=False)

    x_d = nc.dram_tensor("x", [B_PER * NP, C], f32, kind="ExternalInput")
    e_d = nc.dram_tensor("e_wxt", [N, R], f32, kind="ExternalInput")
    f_d = nc.dram_tensor("f_wxt", [N, R], f32, kind="ExternalInput")
    wqt_d = nc.dram_tensor("wqt", [C, C], f32, kind="ExternalInput")
    bq_d = nc.dram_tensor("bq", [C, 1], f32, kind="ExternalInput")
    wkt_d = nc.dram_tensor("wkt", [C, C], f32, kind="ExternalInput")
    wvt_d = nc.dram_tensor("wvt", [C, 256], f32, kind="ExternalInput")
    ckt_d = nc.dram_tensor("const_kt", [C, R], f32, kind="ExternalInput")
    cv_d = nc.dram_tensor("const_v", [R, C], f32, kind="ExternalInput")
    pw_hi_d = nc.dram_tensor("projwt_hi", [128, 256], f32, kind="ExternalInput")
    pw_lo_d = nc.dram_tensor("projwt_lo_aug", [65, 256], f32, kind="ExternalInput")
    ident_d = nc.dram_tensor("ident", [128, 128], f32, kind="ExternalInput")
    ones_d = nc.dram_tensor("ones_att", [128, 32], f32, kind="ExternalInput")
    out_d = nc.dram_tensor("out", [B_PER * NP, C], f32, kind="ExternalOutput")

    NCH = 25  # n-chunks per batch for the E/F contraction (24*128 + 64)

    with TileContext(nc) as tc:
        with tc.tile_pool(name="const", bufs=1) as cpool, \
             tc.tile_pool(name="ef", bufs=1) as efpool, \
             tc.tile_pool(name="low", bufs=1) as lowpool, \
             tc.tile_pool(name="xin", bufs=3) as xpool, \
             tc.tile_pool(name="xt", bufs=2) as xtpool, \
             tc.tile_pool(name="qt", bufs=2) as qtpool, \
             tc.tile_pool(name="sp", bufs=2) as sppool, \
             tc.tile_pool(name="div", bufs=2) as divpool, \
             tc.tile_pool(name="av", bufs=2) as avpool, \
             tc.tile_pool(name="osb", bufs=4) as opool, \
             tc.tile_pool(name="ps", bufs=8, space="PSUM") as ps:

            # ---- constants ----
            ident = cpool.tile([128, 128], f32r)
            nc.sync.dma_start(ident[:], ident_d[:].bitcast(f32r))
            wqt = cpool.tile([128, C], f32r)
            nc.sync.dma_start(wqt[:], wqt_d[0:128, :].bitcast(f32r))
            wqt_l = cpool.tile([64, C], f32r)
            nc.sync.dma_start(wqt_l[:], wqt_d[128:192, :].bitcast(f32r))
            bq_h = cpool.tile([128, 1], f32)
            nc.sync.dma_start(bq_h[:], bq_d[0:128, :])
            bq_l = cpool.tile([64, 1], f32)
            nc.sync.dma_start(bq_l[:], bq_d[128:192, :])
            wkt = cpool.tile([128, C], f32r)
            nc.sync.dma_start(wkt[:], wkt_d[0:128, :].bitcast(f32r))
            wkt_l = cpool.tile([64, C], f32r)
            nc.sync.dma_start(wkt_l[:], wkt_d[128:192, :].bitcast(f32r))
            wvt = cpool.tile([128, 256], f32r)
            nc.sync.dma_start(wvt[:], wvt_d[0:128, :].bitcast(f32r))
            wvt_l = cpool.tile([64, 256], f32r)
            nc.sync.dma_start(wvt_l[:], wvt_d[128:192, :].bitcast(f32r))
            ckt_h = cpool.tile([128, R], f32)
            nc.sync.dma_start(ckt_h[:], ckt_d[0:128, :])
            ckt_l = cpool.tile([64, R], f32)
            nc.sync.dma_start(ckt_l[:], ckt_d[128:192, :])
            cv = cpool.tile([128, C], f32)
            nc.sync.dma_start(cv[:], cv_d[:])
            pw_hi = cpool.tile([128, 256], f32r)
            nc.sync.dma_start(pw_hi[:], pw_hi_d[:].bitcast(f32r))
            pw_lo = cpool.tile([65, 256], f32r)
            nc.sync.dma_start(pw_lo[:], pw_lo_d[:].bitcast(f32r))
            ones_att = cpool.tile([128, 32], bf16)
            nc.gpsimd.dma_start(ones_att[:], ones_d[:])  # f32 -> bf16 cast DMA

            # E/F transposed weights resident in SBUF: 24 full chunks + tail
            e_sb = efpool.tile([128, 24 * 128], f32r)
            f_sb = efpool.tile([128, 24 * 128], f32r)
            for k in range(24):
                nc.sync.dma_start(
                    e_sb[:, k * 128 : (k + 1) * 128],
                    e_d[k * 128 : (k + 1) * 128, :].bitcast(f32r),
                )
                nc.sync.dma_start(
                    f_sb[:, k * 128 : (k + 1) * 128],
                    f_d[k * 128 : (k + 1) * 128, :].bitcast(f32r),
                )
            e_tl = efpool.tile([64, 128], f32r)
            nc.sync.dma_start(e_tl[:], e_d[3072:3136, :].bitcast(f32r))
            f_tl = efpool.tile([64, 128], f32r)
            nc.sync.dma_start(f_tl[:], f_d[3072:3136, :].bitcast(f32r))

            # per-batch low-rank tensors (kept resident across phase B)
            klo_h = [lowpool.tile([128, R], f32r, name=f"klo_h{b}") for b in range(B_PER)]
            klo_l = [lowpool.tile([64, R], f32r, name=f"klo_l{b}") for b in range(B_PER)]
            vlo = [lowpool.tile([128, C], bf16, name=f"vlo{b}") for b in range(B_PER)]

            # ---------------- Phase A: EP/FP + low-rank projections ----------
            for p2 in range(2):
                ep_ps = ps.tile([128, 2 * C], f32, name="ep_ps", tag="bank")
                fp_ps = ps.tile([128, 2 * C], f32, name="fp_ps", tag="bank")
                for ci in range(NCH):
                    nk = 128 if ci < 24 else 64
                    x2 = xpool.tile([nk, 2 * C], f32r, name="x2", tag="x2")
                    for b2 in range(2):
                        off = (2 * p2 + b2) * NP + ci * 128
                        nc.sync.dma_start(
                            x2[:, b2 * C : (b2 + 1) * C],
                            x_d[off : off + nk, :].bitcast(f32r),
                        )
                    elh = e_sb[:, ci * 128 : (ci + 1) * 128] if ci < 24 else e_tl[:]
                    flh = f_sb[:, ci * 128 : (ci + 1) * 128] if ci < 24 else f_tl[:]
                    nc.tensor.matmul(
                        ep_ps[:], elh, x2[:], start=(ci == 0), stop=(ci == NCH - 1)
                    )
                    nc.tensor.matmul(
                        fp_ps[:], flh, x2[:], start=(ci == 0), stop=(ci == NCH - 1)
                    )
                ep_sb = xpool.tile([128, 2 * C], f32r, name="ep_sb", tag="ep_sb")
                nc.vector.tensor_copy(ep_sb[:], ep_ps[:].bitcast(f32r))
                fp_sb = xpool.tile([128, 2 * C], f32r, name="fp_sb", tag="fp_sb")
                nc.vector.tensor_copy(fp_sb[:], fp_ps[:].bitcast(f32r))

                for b2 in range(2):
                    b = 2 * p2 + b2
                    # transpose EP, FP slices: (r=128, c=192) -> (c, r)
                    ept_h = xpool.tile([128, 128], f32r, name="ept_h", tag="ept_h")
                    ept_l = xpool.tile([64, 128], f32r, name="ept_l", tag="ept_l")
                    fpt_h = xpool.tile([128, 128], f32r, name="fpt_h", tag="fpt_h")
                    fpt_l = xpool.tile([64, 128], f32r, name="fpt_l", tag="fpt_l")
                    for (src, dsth, dstl) in ((ep_sb, ept_h, ept_l), (fp_sb, fpt_h, fpt_l)):
                        tp1 = ps.tile([128, 128], f32, name="tp1", tag="bank")
                        nc.tensor.transpose(
                            tp1[:].bitcast(f32r),
                            src[:, b2 * C : b2 * C + 128],
                            ident[:],
                        )
                        nc.vector.tensor_copy(dsth[:], tp1[:].bitcast(f32r))
                        tp2 = ps.tile([64, 128], f32, name="tp2", tag="bank")
                        nc.tensor.transpose(
                            tp2[:].bitcast(f32r),
                            src[:, b2 * C + 128 : b2 * C + 192],
                            ident[:],
                        )
                        nc.vector.tensor_copy(dstl[:], tp2[:].bitcast(f32r))

                    # k_lowT = WkT.T @ EPT + const_kT  (feature-major (kch, r))
                    kl_hi = ps.tile([128, R], f32, name="kl_hi", tag="bank")
                    nc.tensor.matmul(kl_hi[:], wkt[:, 0:128], ept_h[:], start=True, stop=False)
                    nc.tensor.matmul(kl_hi[:], wkt_l[:, 0:128], ept_l[:], start=False, stop=True)
                    nc.vector.tensor_tensor(
                        klo_h[b][:], kl_hi[:], ckt_h[:], op=mybir.AluOpType.add
                    )
                    kl_lo = ps.tile([64, R], f32, name="kl_lo", tag="bank")
                    nc.tensor.matmul(kl_lo[:], wkt[:, 128:192], ept_h[:], start=True, stop=False)
                    nc.tensor.matmul(kl_lo[:], wkt_l[:, 128:192], ept_l[:], start=False, stop=True)
                    nc.vector.tensor_tensor(
                        klo_l[b][:], kl_lo[:], ckt_l[:], op=mybir.AluOpType.add
                    )
                    # v_low (R-major (r, vch)), straight to bf16 with const add
                    vl_ps = ps.tile([128, 256], f32, name="vl_ps", tag="bank")
                    nc.tensor.matmul(vl_ps[:], fpt_h[:], wvt[:], start=True, stop=False)
                    nc.tensor.matmul(vl_ps[:], fpt_l[:], wvt_l[:], start=False, stop=True)
                    nc.vector.tensor_tensor(
                        vlo[b][:], vl_ps[:, 0:C], cv[:], op=mybir.AluOpType.add
                    )

            # ---------------- Phase B: per batch, per token tile ----------
            for b in range(B_PER):
                for t in range(7):
                    W = 512 if t < 6 else 128
                    KCH = W // 128
                    base = b * NP + t * 512
                    x_sb = xpool.tile([128, KCH * C], f32r, name="x_sb", tag="x_sb")
                    for k in range(KCH):
                        nc.sync.dma_start(
                            x_sb[:, k * C : (k + 1) * C],
                            x_d[base + k * 128 : base + (k + 1) * 128, :].bitcast(f32r),
                        )
                    xt_h = xtpool.tile([128, W], f32r, name="xt_h", tag="xt_h")
                    xt_l = xtpool.tile([64, W], f32r, name="xt_l", tag="xt_l")
                    for k in range(KCH):
                        th = ps.tile([128, 128], f32, name="th", tag="bank")
                        nc.tensor.transpose(
                            th[:].bitcast(f32r), x_sb[:, k * C : k * C + 128], ident[:]
                        )
                        nc.vector.tensor_copy(
                            xt_h[:, k * 128 : (k + 1) * 128], th[:].bitcast(f32r)
                        )
                        tl = ps.tile([64, 128], f32, name="tl", tag="bank")
                        nc.tensor.transpose(
                            tl[:].bitcast(f32r),
                            x_sb[:, k * C + 128 : (k + 1) * C],
                            ident[:],
                        )
                        nc.vector.tensor_copy(
                            xt_l[:, k * 128 : (k + 1) * 128], tl[:].bitcast(f32r)
                        )

                    # q projection, feature-major (qch, tok)
                    q_hi = ps.tile([128, W], f32, name="q_hi", tag="bank")
                    nc.tensor.matmul(q_hi[:], wqt[:, 0:128], xt_h[:], start=True, stop=False)
                    nc.tensor.matmul(q_hi[:], wqt_l[:, 0:128], xt_l[:], start=False, stop=True)
                    q_lo = ps.tile([64, W], f32, name="q_lo", tag="bank")
                    nc.tensor.matmul(q_lo[:], wqt[:, 128:192], xt_h[:], start=True, stop=False)
                    nc.tensor.matmul(q_lo[:], wqt_l[:, 128:192], xt_l[:], start=False, stop=True)
                    qt_h = qtpool.tile([128, W], f32r, name="qt_h", tag="qt_h")
                    nc.vector.tensor_scalar(
                        out=qt_h[:], in0=q_hi[:], scalar1=bq_h[:], scalar2=None,
                        op0=mybir.AluOpType.add,
                    )
                    qt_l = qtpool.tile([64, W], f32r, name="qt_l", tag="qt_l")
                    nc.vector.tensor_scalar(
                        out=qt_l[:], in0=q_lo[:], scalar1=bq_l[:], scalar2=None,
                        op0=mybir.AluOpType.add,
                    )

                    # scores: fp32r row-packed (4 heads on k/q hi, 2 on lo)
                    sps = []
                    for h in range(H):
                        s_ps = ps.tile([128, W], f32, name=f"s{h}", tag="bank")
                        if h < 4:
                            nc.tensor.matmul(
                                s_ps[:],
                                klo_h[b][32 * h : 32 * h + 32, :],
                                qt_h[32 * h : 32 * h + 32, :],
                                start=True, stop=True,
                                tile_position=(32 * h, 0),
                            )
                        else:
                            hh = h - 4
                            nc.tensor.matmul(
                                s_ps[:],
                                klo_l[b][32 * hh : 32 * hh + 32, :],
                                qt_l[32 * hh : 32 * hh + 32, :],
                                start=True, stop=True,
                                tile_position=(32 * hh, 0),
                            )
                        sps.append(s_ps)
                    spt = []
                    for h in range(H):
                        sp_t = sppool.tile([128, W], bf16, name=f"sp{h}", tag=f"sp{h}")
                        nc.scalar.activation(
                            sp_t[:], sps[h][:], mybir.ActivationFunctionType.Exp
                        )
                        spt.append(sp_t)

                    # attn @ v_low + denominators, bf16 col-packed
                    avU = ps.tile([128, W], f32, name="avU", tag="bank")
                    for h in range(4):
                        nc.tensor.matmul(
                            avU[32 * h : 32 * h + 32, :],
                            vlo[b][:, 32 * h : 32 * h + 32],
                            spt[h][:],
                            start=True, stop=True,
                            tile_position=(0, 32 * h),
                        )
                    avU2 = ps.tile([64, W], f32, name="avU2", tag="bank")
                    for h in range(4, 6):
                        hh = h - 4
                        nc.tensor.matmul(
                            avU2[32 * hh : 32 * hh + 32, :],
                            vlo[b][:, 32 * h : 32 * h + 32],
                            spt[h][:],
                            start=True, stop=True,
                            tile_position=(0, 32 * hh),
                        )
                    z1 = ps.tile([128, W], f32, name="z1", tag="bank")
                    for h in range(4):
                        nc.tensor.matmul(
                            z1[32 * h : 32 * h + 32, :],
                            ones_att[:],
                            spt[h][:],
                            start=True, stop=True,
                            tile_position=(0, 32 * h),
                        )
                    z2 = ps.tile([64, W], f32, name="z2", tag="bank")
                    for h in range(4, 6):
                        hh = h - 4
                        nc.tensor.matmul(
                            z2[32 * hh : 32 * hh + 32, :],
                            ones_att[:],
                            spt[h][:],
                            start=True, stop=True,
                            tile_position=(0, 32 * hh),
                        )
                    rz1 = divpool.tile([128, W], f32, name="rz1", tag="rz1")
                    nc.vector.reciprocal(rz1[:], z1[:])
                    rz2 = divpool.tile([64, W], f32, name="rz2", tag="rz2")
                    nc.vector.reciprocal(rz2[:], z2[:])
                    av_hi = avpool.tile([128, W], f32r, name="av_hi", tag="av_hi")
                    nc.vector.tensor_tensor(
                        av_hi[:], avU[:], rz1[:], op=mybir.AluOpType.mult
                    )
                    av_lo = avpool.tile([65, W], f32r, name="av_lo", tag="av_lo")
                    nc.vector.tensor_tensor(
                        av_lo[0:64, :], avU2[:], rz2[:], op=mybir.AluOpType.mult
                    )
                    nc.vector.tensor_scalar(
                        out=av_lo[64:65, :], in0=xt_h[0:1, :], scalar1=0.0, scalar2=1.0,
                        op0=mybir.AluOpType.mult, op1=mybir.AluOpType.add,
                    )

                    # proj: out token-major (tok, co), K = 128 + 65(bias row)
                    for m in range(KCH):
                        p_ps = ps.tile([128, 256], f32, name="p_ps", tag="bank")
                        nc.tensor.matmul(
                            p_ps[:],
                            av_hi[:, m * 128 : (m + 1) * 128],
                            pw_hi[:],
                            start=True, stop=False,
                        )
                        nc.tensor.matmul(
                            p_ps[:],
                            av_lo[:, m * 128 : (m + 1) * 128],
                            pw_lo[:],
                            start=False, stop=True,
                        )
                        o_sb = opool.tile([128, C], f32, name="o_sb", tag="o_sb")
                        nc.vector.tensor_copy(o_sb[:], p_ps[:, 0:C])
                        nc.sync.dma_start(
                            out_d[base + m * 128 : base + (m + 1) * 128, :], o_sb[:]
                        )

    nc.compile()
    return nc


def _get_nc():
    if "nc" not in _STATE:
        _STATE["nc"] = _build_bass()
    return _STATE["nc"]


def kernel(x, qkv_w, qkv_b, E_w, E_b, F_w, F_b, proj_w, proj_b, h, w):
    from concourse.bass_utils import run_bass_kernel_spmd

    x = np.asarray(x, dtype=np.float32)
    qkv_w = np.asarray(qkv_w, dtype=np.float32)
    qkv_b = np.asarray(qkv_b, dtype=np.float32)
    E_w = np.asarray(E_w, dtype=np.float32)
    E_b = np.asarray(E_b, dtype=np.float32)
    F_w = np.asarray(F_w, dtype=np.float32)
    F_b = np.asarray(F_b, dtype=np.float32)
    proj_w = np.asarray(proj_w, dtype=np.float32)
    proj_b = np.asarray(proj_b, dtype=np.float32)
    assert int(h) == 56 and int(w) == 56

    n_of_m = _window_perm()
    E_wx = np.ascontiguousarray(E_w[:, n_of_m])
    F_wx = np.ascontiguousarray(F_w[:, n_of_m])

    Wq, Wk, Wv = qkv_w[0:C], qkv_w[C : 2 * C], qkv_w[2 * C : 3 * C]
    bq, bk, bv = qkv_b[0:C], qkv_b[C : 2 * C], qkv_b[2 * C : 3 * C]
    scale = np.float32(1.0 / np.sqrt(HD))

    const_k = np.outer(E_wx.sum(1), bk) + E_b[:, None]      # (128, 192)
    const_v = (np.outer(F_wx.sum(1), bv) + F_b[:, None]).astype(np.float32)

    wqt = np.ascontiguousarray((Wq * scale).T)              # (192, 192)
    bq_s = np.ascontiguousarray((bq * scale).reshape(C, 1))
    wkt = np.ascontiguousarray(Wk.T)
    wvt = np.zeros((C, 256), dtype=np.float32)
    wvt[:, 0:C] = Wv.T
    ckt = np.ascontiguousarray(const_k.T.astype(np.float32))  # (192, 128)
    pw = proj_w.T                                            # (ch, co)
    pw_hi = np.zeros((128, 256), dtype=np.float32)
    pw_hi[:, 0:C] = pw[0:128]
    pw_lo = np.zeros((65, 256), dtype=np.float32)
    pw_lo[0:64, 0:C] = pw[128:192]
    pw_lo[64, 0:C] = proj_b

    e_wxt = np.ascontiguousarray(E_wx.T)                     # (3136, 128)
    f_wxt = np.ascontiguousarray(F_wx.T)
    ident = np.eye(128, dtype=np.float32)
    ones_att = np.ones((128, 32), dtype=np.float32)

    consts = dict(
        e_wxt=e_wxt, f_wxt=f_wxt, wqt=wqt, bq=bq_s, wkt=wkt, wvt=wvt,
        const_kt=ckt, const_v=const_v, projwt_hi=pw_hi, projwt_lo_aug=pw_lo,
        ident=ident, ones_att=ones_att,
    )

    # shard x: core i gets batches 4i..4i+4, padded to NP tokens per batch
    xb = x.reshape(B_TOT, 64 * 49, C)
    in_maps = []
    for i in range(N_CORES):
        xi = np.zeros((B_PER, NP, C), dtype=np.float32)
        xi[:, 0:N, :] = xb[B_PER * i : B_PER * (i + 1)]
        in_maps.append({**consts, "x": xi.reshape(B_PER * NP, C)})

    nc = _get_nc()
    _STATE["last_in_maps"] = in_maps
    res = run_bass_kernel_spmd(nc, in_maps, core_ids=list(range(N_CORES)))

    out_win = np.empty((B_TOT, N, C), dtype=np.float32)
    for i in range(N_CORES):
        oi = res.results[i]["out"].reshape(B_PER, NP, C)
        out_win[B_PER * i : B_PER * (i + 1)] = oi[:, 0:N, :]
    # window_reverse on the gathered output
    out_sp = (
        out_win.reshape(B_TOT, 8, 8, 7, 7, C)
        .transpose(0, 1, 3, 2, 4, 5)
        .reshape(B_TOT, N, C)
    )
    return np.ascontiguousarray(out_sp)
